# revision 17
# baseline (speedup 1.0000x reference)
"""DyDCNv2 (modulated deformable conv 3x3 + GroupNorm) on 8 Trainium2 cores.

Sharding: core c handles batch b=c//2, row-half h=c%2 (48 of 96 rows).
Per core the full per-batch input image is kept in DRAM as a row-major
[H*W, C] table; bilinear corners are fetched with SWDGE dma_gather
(1 descriptor per (pixel, tap, y-side), each covering the x0/x0+1 pair of
channel vectors), blended on DVE with per-pixel corner weights (mask and
zero-pad validity folded in), transposed tile-wise on the PE to [cin, pix],
and accumulated over the 9 taps into PSUM as [cout, pix] matmuls.
GroupNorm statistics are reduced across the core pair sharing a batch with
a tiny AllReduce, then applied in place before the single output store.

Pixel layout convention ("gather layout"): local pixel i = slot*128 + p
(p = SBUF partition). Corner weights are computed elementwise in a
DMA-friendly transposed layout [36, 9, 128] and PE-transposed into place;
gather indices likewise in [96, 9, 3, 16] then PE-transposed into the
int16 [16, n/16] wrap layout dma_gather wants.
"""

import os
import numpy as np
import ml_dtypes

import concourse.bass as bass
import concourse.bacc as bacc
import concourse.mybir as mybir
import concourse.tile as tile
from concourse import library_config
from concourse.bass_utils import run_bass_kernel_spmd

P = 128
B, CIN, COUT, H, W = 4, 256, 256, 96, 96
HP = H // 2              # output rows per core
NPIX = HP * W            # 4608 output pixels per core
NT = 9                   # 3x3 taps
R = H * W + 4            # gather table rows (1 front + 3 back pad; idx is shifted +1)
NCH = 3                  # pixel chunks per core
CH = NPIX // NCH         # 1536 pixels per chunk
NSLOT = CH // P          # 12
NSUB = CH // 512         # 3 matmul sub-tiles of 512
NJ = NPIX // P           # 36 gather-layout columns per tap
NM = NPIX // 48          # 96 idx-prelayout partitions
GN_GROUPS = 16
EPS = 1e-5
NG = (CIN // GN_GROUPS) * (H * W)  # elements per GN group (full image)

F32 = mybir.dt.float32
BF16 = mybir.dt.bfloat16
I16 = mybir.dt.int16
AX = mybir.AxisListType
OP = mybir.AluOpType

USE_BF16 = os.environ.get("DCN_DTYPE", "bf16") != "fp32"
NOGATHER = os.environ.get("DCN_NOGATHER") == "1"
DTM = BF16 if USE_BF16 else F32
NPDT = ml_dtypes.bfloat16 if USE_BF16 else np.float32

_CACHED = {}


def _build_nc():
    nc = bacc.Bacc("TRN2", target_bir_lowering=False, debug=False, num_devices=8)

    xt = nc.dram_tensor("xt", [R * CIN], DTM, kind="ExternalInput")
    offs = nc.dram_tensor("offs", [18, NPIX], F32, kind="ExternalInput")
    msk = nc.dram_tensor("msk", [NT, NPIX], F32, kind="ExternalInput")
    wtd = nc.dram_tensor("wtd", [NT, CIN, COUT], DTM, kind="ExternalInput")
    # base sampling grids incl. tap deltas, in the two compute layouts
    bw_y = nc.dram_tensor("bw_y", [NJ, NT, P], F32, kind="ExternalInput")
    bw_x = nc.dram_tensor("bw_x", [NJ, NT, P], F32, kind="ExternalInput")
    bi_y = nc.dram_tensor("bi_y", [NM, NT, NCH, 16], F32, kind="ExternalInput")
    bi_x = nc.dram_tensor("bi_x", [NM, NT, NCH, 16], F32, kind="ExternalInput")
    identd = nc.dram_tensor("identd", [P, P], F32, kind="ExternalInput")
    ind8d = nc.dram_tensor("ind8d", [P, 8], F32, kind="ExternalInput")
    e8d = nc.dram_tensor("e8d", [8, P], F32, kind="ExternalInput")
    gamd = nc.dram_tensor("gamd", [P, 2], F32, kind="ExternalInput")
    betd = nc.dram_tensor("betd", [P, 2], F32, kind="ExternalInput")
    yout = nc.dram_tensor("y", [COUT, NPIX], F32, kind="ExternalOutput")
    DEBUG = os.environ.get("DCN_DEBUG") == "1"
    if DEBUG:
        dbg_idx = [nc.dram_tensor(f"dbg_idx{s_}", [P, NT, NCH, CH // 16], I16,
                                  kind="ExternalOutput") for s_ in range(4)]
        dbg_w = [nc.dram_tensor(f"dbg_w{k}", [P, NT, NJ], DTM,
                                kind="ExternalOutput") for k in range(4)]
        dbg_y = nc.dram_tensor("dbg_y", [P, 2, NPIX], F32, kind="ExternalOutput")

    nc.gpsimd.load_library(library_config.mlp)

    gather_src = bass.AP(xt[:].tensor, 0, [[CIN, R], [1, CIN]])

    with tile.TileContext(nc) as tc:
        with (
            tc.tile_pool(name="const", bufs=1) as cp,
            tc.tile_pool(name="dram", bufs=1, space="DRAM") as dp,
        ):
            # ---------------- constants ----------------
            wtile = cp.tile([P, NT, 2, COUT], DTM, tag="wtile")
            nc.sync.dma_start(wtile[:], wtd.rearrange("t (kc p) o -> p t kc o", p=P))
            identf = cp.tile([P, P], F32, tag="identf")
            nc.sync.dma_start(identf[:], identd[:])
            ident = cp.tile([P, P], DTM, tag="ident")
            nc.vector.tensor_copy(ident[:], identf[:])
            ind8 = cp.tile([P, 8], F32, tag="ind8")
            nc.sync.dma_start(ind8[:], ind8d[:])
            e8 = cp.tile([8, P], F32, tag="e8")
            nc.sync.dma_start(e8[:], e8d[:])
            gam = cp.tile([P, 2], F32, tag="gam")
            nc.sync.dma_start(gam[:], gamd[:])
            bet = cp.tile([P, 2], F32, tag="bet")
            nc.sync.dma_start(bet[:], betd[:])

            # persistent setup outputs
            wc = [cp.tile([P, NT, NJ], DTM, tag=f"wb{k}", name=f"wb{k}")
                  for k in range(4)]  # corner weights [y0x0, y0x1, y1x0, y1x1]
            idxs = [cp.tile([P, NT, NCH, CH // 16], I16, tag=f"idx{k}", name=f"idx{k}")
                    for k in range(4)]  # [A+0, A+1, B+0, B+1]
            ysb = cp.tile([P, 2, NPIX], F32, tag="ysb")

            # ---------------- setup: weights + indices ----------------
            with (
                tc.tile_pool(name="setup", bufs=1) as wk,
                tc.tile_pool(name="spsum", bufs=2, space="PSUM") as spp,
            ):
                # --- corner weights, computed in [NJ, NT, P] layout ---
                offw = offs.rearrange("(t two) (j p) -> two j t p", two=2, p=P)

                def build_cw(base_d, off_idx, lim):
                    pyt = wk.tile([NJ, NT, P], F32, tag=f"py{off_idx}")
                    nc.sync.dma_start(pyt[:], offw[off_idx])
                    bt = wk.tile([NJ, NT, P], F32, tag=f"b{off_idx}")
                    nc.sync.dma_start(bt[:], base_d[:])
                    nc.vector.tensor_tensor(pyt[:], pyt[:], bt[:], op=OP.add)
                    fy = wk.tile([NJ, NT, P], F32, tag=f"f{off_idx}")
                    gt = wk.tile([NJ, NT, P], F32, tag=f"g{off_idx}")
                    # floor via round-to-nearest magic + compare correction
                    nc.vector.tensor_scalar(fy[:], pyt[:], 12582912.0, -12582912.0,
                                            op0=OP.add, op1=OP.add)
                    nc.vector.tensor_tensor(gt[:], fy[:], pyt[:], op=OP.is_gt)
                    y0 = bt  # reuse
                    nc.vector.tensor_tensor(y0[:], fy[:], gt[:], op=OP.subtract)
                    nc.vector.tensor_tensor(fy[:], pyt[:], y0[:], op=OP.subtract)
                    w0 = pyt  # reuse: w0 = 1 - f
                    nc.vector.tensor_scalar(w0[:], fy[:], -1.0, 1.0, op0=OP.mult, op1=OP.add)
                    v0 = wk.tile([NJ, NT, P], F32, tag=f"v0{off_idx}")
                    vtmp = wk.tile([NJ, NT, P], F32, tag=f"vt{off_idx}")
                    nc.vector.tensor_scalar(v0[:], y0[:], 0.0, None, op0=OP.is_ge)
                    nc.vector.tensor_scalar(vtmp[:], y0[:], float(lim - 1), None, op0=OP.is_le)
                    nc.vector.tensor_tensor(v0[:], v0[:], vtmp[:], op=OP.mult)
                    v1 = wk.tile([NJ, NT, P], F32, tag=f"v1{off_idx}")
                    nc.vector.tensor_scalar(v1[:], y0[:], -1.0, None, op0=OP.is_ge)
                    nc.vector.tensor_scalar(vtmp[:], y0[:], float(lim - 2), None, op0=OP.is_le)
                    nc.vector.tensor_tensor(v1[:], v1[:], vtmp[:], op=OP.mult)
                    nc.vector.tensor_tensor(v0[:], w0[:], v0[:], op=OP.mult)
                    nc.vector.tensor_tensor(v1[:], fy[:], v1[:], op=OP.mult)
                    return v0, v1

                cy0, cy1 = build_cw(bw_y, 0, H)
                cx0, cx1 = build_cw(bw_x, 1, W)
                mskt = wk.tile([NJ, NT, P], F32, tag="mskt")
                nc.sync.dma_start(mskt[:], msk.rearrange("t (j p) -> j t p", p=P))

                for k, (cy, cx) in enumerate(
                        ((cy0, cx0), (cy0, cx1), (cy1, cx0), (cy1, cx1))):
                    wf = wk.tile([NJ, NT, P], F32, tag="wf")
                    nc.vector.tensor_tensor(wf[:], cy[:], cx[:], op=OP.mult)
                    nc.vector.tensor_tensor(wf[:], wf[:], mskt[:], op=OP.mult)
                    for t in range(NT):
                        pw = spp.tile([P, NJ], F32, tag="pw")
                        nc.tensor.transpose(pw[:], wf[:, t, :], identf[:NJ, :NJ])
                        nc.scalar.copy(out=wc[k][:, t, :], in_=pw[:])

                # --- indices, computed in [NM, NT, NCH, 16] layout ---
                offi = offs.rearrange("(t two) (c m q) -> two m t c q", two=2, q=16, m=NM)

                def build_floor16(base_d, off_idx):
                    pt = wk.tile([NM, NT, NCH, 16], F32, tag=f"p6{off_idx}")
                    for c in range(NCH):
                        nc.sync.dma_start(pt[:, :, c, :], offi[off_idx][:, :, c, :])
                    bt = wk.tile([NM, NT, NCH, 16], F32, tag=f"b6{off_idx}")
                    nc.sync.dma_start(bt[:], base_d[:])
                    nc.vector.tensor_tensor(pt[:], pt[:], bt[:], op=OP.add)
                    ft = wk.tile([NM, NT, NCH, 16], F32, tag=f"f6{off_idx}")
                    gt = wk.tile([NM, NT, NCH, 16], F32, tag=f"g6{off_idx}")
                    nc.vector.tensor_scalar(ft[:], pt[:], 12582912.0, -12582912.0,
                                            op0=OP.add, op1=OP.add)
                    nc.vector.tensor_tensor(gt[:], ft[:], pt[:], op=OP.is_gt)
                    nc.vector.tensor_tensor(pt[:], ft[:], gt[:], op=OP.subtract)
                    return pt

                y06 = build_floor16(bi_y, 0)
                x06 = build_floor16(bi_x, 1)
                xb = x06
                nc.vector.tensor_scalar(xb[:], x06[:], 0.0, float(W + 1), op0=OP.max, op1=OP.min)

                for side in range(2):
                    yc = wk.tile([NM, NT, NCH, 16], F32, tag="yc")
                    if side == 0:
                        nc.vector.tensor_scalar(yc[:], y06[:], 0.0, float(H - 1),
                                                op0=OP.max, op1=OP.min)
                    else:
                        nc.vector.tensor_scalar(yc[:], y06[:], 1.0, None, op0=OP.add)
                        nc.vector.tensor_scalar(yc[:], yc[:], 0.0, float(H - 1),
                                                op0=OP.max, op1=OP.min)
                    nc.vector.tensor_scalar(yc[:], yc[:], float(W), None, op0=OP.mult)
                    nc.vector.tensor_tensor(yc[:], yc[:], xb[:], op=OP.add)
                    nc.vector.tensor_scalar(yc[:], yc[:], 0.0, float(H * W + 2),
                                            op0=OP.max, op1=OP.min)
                    for xi in range(2):
                        k = 2 * side + xi
                        if xi == 1:
                            nc.vector.tensor_scalar(yc[:], yc[:], 1.0, None, op0=OP.add)
                        nc.vector.memset(idxs[k][:], 0)
                        for t in range(NT):
                            for c in range(NCH):
                                pi = spp.tile([16, NM], F32, tag="pi")
                                nc.tensor.transpose(pi[:], yc[:, t, c, :],
                                                    identf[:NM, :NM])
                                nc.vector.tensor_copy(idxs[k][:16, t, c, :], pi[:])
                        # the gather ucode reads per-Q7-core replicas from each
                        # 16-partition group: bounce through DRAM and replicate
                        ib = dp.tile([16, NT * NCH * (CH // 16)], I16, tag=f"ib{k}",
                                     name=f"ib{k}")
                        nc.sync.dma_start(ib[:], idxs[k][:16])
                        for g in range(1, 8):
                            nc.sync.dma_start(idxs[k][16 * g:16 * (g + 1)], ib[:])

            if DEBUG:
                for s_ in range(4):
                    nc.sync.dma_start(dbg_idx[s_][:], idxs[s_][:])
                for k in range(4):
                    nc.sync.dma_start(dbg_w[k][:], wc[k][:])

            # ---------------- main conv loop ----------------
            with (
                tc.tile_pool(name="gat", bufs=2) as gp,
                tc.tile_pool(name="blend", bufs=2) as bp,
                tc.tile_pool(name="vt", bufs=2) as vp,
                tc.tile_pool(name="acc", bufs=1, space="PSUM") as accp,
                tc.tile_pool(name="pt", bufs=2, space="PSUM") as ptp,
            ):
                for c in range(NCH):
                    ps = [accp.tile([P, CH], F32, tag=f"acc{cc}", name=f"acc{cc}")
                          for cc in range(2)]
                    for t in range(NT):
                        G = []
                        for k in range(4):
                            g = gp.tile([P, NSLOT, CIN], DTM, tag=f"G{k}",
                                        name=f"G{k}")
                            if NOGATHER:
                                nc.vector.memset(g[:], 0.5)
                            else:
                                nc.gpsimd.dma_gather(
                                    g[:], gather_src,
                                    idxs[k][:, t, c, :],
                                    CH, CH, CIN, single_packet=False,
                                )
                            G.append(g)
                        # gather k order: [A+0, A+1, B+0, B+1] = corners
                        # [y0x0, y0x1, y1x0, y1x1] = weight order

                        def wsl(k):
                            return wc[k][:, t, c * NSLOT:(c + 1) * NSLOT].to_broadcast(
                                [P, NSLOT, CIN])

                        m0 = bp.tile([P, NSLOT, CIN], DTM, tag="m0")
                        m1 = bp.tile([P, NSLOT, CIN], DTM, tag="m1")
                        m2 = bp.tile([P, NSLOT, CIN], DTM, tag="m2")
                        nc.vector.tensor_tensor(m0[:], G[0][:], wsl(0), op=OP.mult)
                        nc.vector.tensor_tensor(m1[:], G[1][:], wsl(1), op=OP.mult)
                        nc.vector.tensor_tensor(m0[:], m0[:], m1[:], op=OP.add)
                        nc.vector.tensor_tensor(m2[:], G[2][:], wsl(2), op=OP.mult)
                        nc.vector.tensor_tensor(m1[:], G[3][:], wsl(3), op=OP.mult)
                        nc.vector.tensor_tensor(m2[:], m2[:], m1[:], op=OP.add)
                        nc.vector.tensor_tensor(m0[:], m0[:], m2[:], op=OP.add)

                        vT = vp.tile([P, 2, CH], DTM, tag="vT")
                        for s in range(NSLOT):
                            for kc in range(2):
                                pst = ptp.tile([P, P], DTM, tag="pt")
                                nc.tensor.transpose(
                                    pst[:], m0[:, s, kc * P:(kc + 1) * P], ident[:])
                                nc.scalar.copy(
                                    out=vT[:, kc, s * P:(s + 1) * P], in_=pst[:])

                        for cc in range(2):
                            for kc in range(2):
                                for sub in range(NSUB):
                                    nc.tensor.matmul(
                                        ps[cc][:, sub * 512:(sub + 1) * 512],
                                        wtile[:, t, kc, cc * P:(cc + 1) * P],
                                        vT[:, kc, sub * 512:(sub + 1) * 512],
                                        start=(t == 0 and kc == 0),
                                        stop=(t == NT - 1 and kc == 1),
                                    )
                    for cc in range(2):
                        nc.scalar.copy(out=ysb[:, cc, c * CH:(c + 1) * CH], in_=ps[cc][:])

            if DEBUG:
                nc.sync.dma_start(dbg_y[:], ysb[:])

            # ---------------- GroupNorm ----------------
            with tc.tile_pool(name="gnp", bufs=1, space="PSUM") as gnp, \
                 tc.tile_pool(name="gns", bufs=1) as wk:
                st = wk.tile([P, 4], F32, tag="st")
                sq = wk.tile([P, NPIX], F32, tag="sq")
                for cc in range(2):
                    nc.vector.reduce_sum(st[:, 2 * cc:2 * cc + 1], ysb[:, cc, :], axis=AX.X)
                    nc.vector.tensor_tensor(sq[:], ysb[:, cc, :], ysb[:, cc, :], op=OP.mult)
                    nc.vector.reduce_sum(st[:, 2 * cc + 1:2 * cc + 2], sq[:], axis=AX.X)
                pg = gnp.tile([8, 4], F32, tag="pg")
                nc.tensor.matmul(pg[:], ind8[:], st[:], start=True, stop=True)
                gsb = wk.tile([8, 4], F32, tag="gsb")
                nc.vector.tensor_copy(gsb[:], pg[:])

                cind = dp.tile([8, 4], F32, tag="cind")
                cout_ = dp.tile([8, 4], F32, tag="cout")
                nc.gpsimd.dma_start(cind[:], gsb[:])
                nc.gpsimd.collective_compute(
                    "AllReduce", OP.add,
                    replica_groups=[[0, 1], [2, 3], [4, 5], [6, 7]],
                    ins=[cind.opt()], outs=[cout_.opt()],
                )
                nc.sync.dma_start(gsb[:], cout_[:])

                mu = wk.tile([8, 2], F32, tag="mu")
                e2 = wk.tile([8, 2], F32, tag="e2")
                nc.vector.tensor_scalar(mu[:], gsb[:, 0::2], 1.0 / NG, None, op0=OP.mult)
                nc.vector.tensor_scalar(e2[:], gsb[:, 1::2], 1.0 / NG, None, op0=OP.mult)
                m2t = wk.tile([8, 2], F32, tag="m2t")
                nc.vector.tensor_tensor(m2t[:], mu[:], mu[:], op=OP.mult)
                nc.vector.tensor_tensor(e2[:], e2[:], m2t[:], op=OP.subtract)
                nc.vector.tensor_scalar(e2[:], e2[:], EPS, None, op0=OP.add)
                rs = wk.tile([8, 2], F32, tag="rs")
                nc.scalar.activation(rs[:], e2[:], mybir.ActivationFunctionType.Sqrt)
                nc.vector.reciprocal(rs[:], rs[:])

                pex = gnp.tile([P, 2], F32, tag="pex")
                rsc = wk.tile([P, 2], F32, tag="rsc")
                nc.tensor.matmul(pex[:], e8[:], rs[:], start=True, stop=True)
                nc.vector.tensor_copy(rsc[:], pex[:])
                pex2 = gnp.tile([P, 2], F32, tag="pex2")
                muc = wk.tile([P, 2], F32, tag="muc")
                nc.tensor.matmul(pex2[:], e8[:], mu[:], start=True, stop=True)
                nc.vector.tensor_copy(muc[:], pex2[:])

                sc = wk.tile([P, 2], F32, tag="sc")
                nc.vector.tensor_tensor(sc[:], rsc[:], gam[:], op=OP.mult)
                sh = wk.tile([P, 2], F32, tag="sh")
                nc.vector.tensor_tensor(sh[:], muc[:], sc[:], op=OP.mult)
                nc.vector.tensor_tensor(sh[:], bet[:], sh[:], op=OP.subtract)

                for cc in range(2):
                    nc.vector.tensor_scalar(
                        ysb[:, cc, :], ysb[:, cc, :],
                        sc[:, cc:cc + 1], sh[:, cc:cc + 1],
                        op0=OP.mult, op1=OP.add)

            nc.sync.dma_start(yout.rearrange("(cc p) i -> p cc i", p=P), ysb[:])

    nc.compile()
    return nc


def _host_pack(x, offset, mask, weight, gamma, beta):
    """Build the 8 per-core input maps (pure layout work)."""
    in_maps = []
    wts = np.ascontiguousarray(
        weight.reshape(COUT, CIN, 9).transpose(2, 1, 0)).astype(NPDT)  # [9, cin, cout]
    ident = np.eye(P, dtype=np.float32)
    pgrid = np.arange(P)
    ind8 = (pgrid[:, None] // 16 == np.arange(8)[None, :]).astype(np.float32)
    e8 = np.ascontiguousarray(ind8.T)
    gam2 = np.ascontiguousarray(gamma.reshape(2, P).T).astype(np.float32)
    bet2 = np.ascontiguousarray(beta.reshape(2, P).T).astype(np.float32)

    # pixel index arrays for the two compute layouts
    i_w = np.arange(NPIX).reshape(NJ, P)          # [36, 128]: pix = j*128+p
    i_i = np.arange(NPIX).reshape(NCH, NM, 16)    # [3, 96, 16]: pix = c*1536+m*16+q
    i_i = i_i.transpose(1, 0, 2)                  # [96, 3, 16]
    dy = (np.arange(NT) // 3 - 1).astype(np.float32)
    dx = (np.arange(NT) % 3 - 1).astype(np.float32)

    def grids(ii, h):
        yy = (h * HP + ii // W).astype(np.float32)
        xx = (ii % W).astype(np.float32)
        # insert tap axis after the leading axis
        gy = np.expand_dims(yy, 1) + dy.reshape((1, NT) + (1,) * (ii.ndim - 1))
        gx = np.expand_dims(xx, 1) + dx.reshape((1, NT) + (1,) * (ii.ndim - 1))
        return (np.ascontiguousarray(gy.astype(np.float32)),
                np.ascontiguousarray(gx.astype(np.float32)))

    for core in range(8):
        b, h = core // 2, core % 2
        xt = np.zeros((R, CIN), dtype=NPDT)
        xt[1:1 + H * W] = x[b].reshape(CIN, H * W).T.astype(NPDT)
        offs = np.ascontiguousarray(
            offset[b, :, h * HP:(h + 1) * HP, :].reshape(18, NPIX)).astype(np.float32)
        mk = np.ascontiguousarray(
            mask[b, :, h * HP:(h + 1) * HP, :].reshape(NT, NPIX)).astype(np.float32)
        bw_y, bw_x = grids(i_w, h)
        bi_y, bi_x = grids(i_i, h)
        bi_x = bi_x + 1.0  # front-pad row: gather indices are shifted by +1

        in_maps.append({
            "xt": xt.reshape(-1),
            "offs": offs,
            "msk": mk,
            "wtd": wts,
            "bw_y": bw_y, "bw_x": bw_x,
            "bi_y": bi_y, "bi_x": bi_x,
            "identd": ident,
            "ind8d": ind8,
            "e8d": e8,
            "gamd": gam2,
            "betd": bet2,
        })
    return in_maps


def kernel(x, offset, mask, weight, gamma, beta):
    x = np.asarray(x, dtype=np.float32)
    offset = np.asarray(offset, dtype=np.float32)
    mask = np.asarray(mask, dtype=np.float32)
    weight = np.asarray(weight, dtype=np.float32)
    gamma = np.asarray(gamma, dtype=np.float32)
    beta = np.asarray(beta, dtype=np.float32)

    if "nc" not in _CACHED:
        _CACHED["nc"] = _build_nc()
    nc = _CACHED["nc"]

    in_maps = _host_pack(x, offset, mask, weight, gamma, beta)
    res = run_bass_kernel_spmd(nc, in_maps, core_ids=list(range(8)))
    _CACHED["last_results"] = res

    out = np.empty((B, COUT, H, W), dtype=np.float32)
    for core in range(8):
        b, h = core // 2, core % 2
        out[b, :, h * HP:(h + 1) * HP, :] = res.results[core]["y"].reshape(COUT, HP, W)
    return out
